# revision 4
# baseline (speedup 1.0000x reference)
"""Trainium2 Bass kernel for a 3-layer GCN + mean-pool + MLP head (8 NeuronCores).

Strategy:
  - shard graphs (and their contiguous node ranges) across 8 cores
  - per layer: g = dinv * (h @ W) produced per-core, exchanged via 4 chunked
    AllGathers into a replicated table; aggregation = dma_gather of 256B rows
    with a per-(core,window) degree-sorted prefix-slot schedule; window
    partials merged by 4 small local gathers; W3 applied post-aggregation
  - pooling via PE matmul with a host-built (1/count) one-hot matrix
"""
import math
import sys
from contextlib import ExitStack

sys.path.insert(0, "/opt/trn_rl_repo")

import numpy as np

import concourse.bass as bass
import concourse.bacc as bacc
import concourse.mybir as mybir
import concourse.tile as tile
from concourse.bass_utils import run_bass_kernel_spmd
from concourse.masks import make_identity



P = 128
NCORES = 8
D = 64          # gather row width (f32) = 256B
CHUNK_COLS = 64   # gather chunk size in columns (64*128 = 8192 idxs)


def _ceil(a, b):
    return -(-a // b)


def build_schedule(edge_index, batch, n_graphs=1024):
    """All host-side preprocessing. Returns dict of constants + per-core arrays."""
    src_g = np.asarray(edge_index[0], dtype=np.int64)
    dst_g = np.asarray(edge_index[1], dtype=np.int64)
    batch = np.asarray(batch, dtype=np.int64)
    n_nodes = batch.shape[0]
    GPC = n_graphs // NCORES

    bounds = np.searchsorted(batch, np.arange(NCORES + 1) * GPC)
    n_real = np.diff(bounds)
    QC = 128 * _ceil(_ceil(int(n_real.max()), 4), 128)
    NMAX = 4 * QC
    NMAXC = NMAX // P
    SSTRIDE = QC + 128
    WROWS = 8 * SSTRIDE
    assert WROWS <= 32768

    # source node -> (owner core, local n) -> window + window-local row
    owner = np.searchsorted(bounds, src_g, side="right") - 1  # [E]
    loc = src_g - bounds[owner]
    q_src = loc // QC
    r_src = loc % QC
    src_window = q_src.astype(np.int64)                      # [E] 0..3
    src_wrow = owner * SSTRIDE + r_src                       # window-local row

    # per-core edge partitions by dst
    dst_owner = np.searchsorted(bounds, dst_g, side="right") - 1
    deg_tot_g = np.bincount(dst_g, minlength=n_nodes)

    cores = []
    for c in range(NCORES):
        m = dst_owner == c
        e_dst = (dst_g[m] - bounds[c]).astype(np.int64)   # local dst
        e_w = src_window[m]
        e_srow = src_wrow[m]
        deg_w = np.zeros((4, NMAX), np.int64)
        for w in range(4):
            np.add.at(deg_w[w], e_dst[e_w == w], 1)
        cores.append(dict(e_dst=e_dst, e_w=e_w, e_srow=e_srow, deg_w=deg_w))

    # per-(core, window) orderings and slot counts
    for c in range(NCORES):
        cc = cores[c]
        cc["order"] = []
        cc["rank"] = []
        cc["slotcnt"] = []
        for w in range(4):
            dw = cc["deg_w"][w]
            order = np.argsort(-dw, kind="stable")
            rank = np.empty(NMAX, np.int64)
            rank[order] = np.arange(NMAX)
            cc["order"].append(order)
            cc["rank"].append(rank)
            kmax = int(dw.max())
            cnt = np.array([(dw > k).sum() for k in range(kmax)], np.int64)
            cc["slotcnt"].append(cnt)

    # union slot schedule per window: CK[w][k] in columns
    CK = []
    for w in range(4):
        kmax = max(len(cores[c]["slotcnt"][w]) for c in range(NCORES))
        cols = []
        for k in range(kmax):
            mx = max(
                int(cores[c]["slotcnt"][w][k]) if k < len(cores[c]["slotcnt"][w]) else 0
                for c in range(NCORES)
            )
            cols.append(NMAXC if k == 0 else _ceil(mx, P))
        CK.append(cols)

    # window stream layout: slot k occupies columns [slot_off[w][k], +CK[w][k])
    slot_off = []
    WCOLS = []
    for w in range(4):
        off = np.concatenate([[0], np.cumsum(CK[w])]).astype(np.int64)
        slot_off.append(off)
        WCOLS.append(int(off[-1]))

    # chunk list per window: (col_off, ncols, segments)
    # segment: (scr_col, acc_col, ncols, is_copy)  [is_copy when slot==0 segment]
    chunks = []
    for w in range(4):
        col = 0
        while col < WCOLS[w]:
            nc_ = min(CHUNK_COLS, WCOLS[w] - col)
            segs = []
            # which slots intersect [col, col+nc_)
            for k in range(len(CK[w])):
                a = max(col, int(slot_off[w][k]))
                b = min(col + nc_, int(slot_off[w][k + 1]))
                if a < b:
                    segs.append((a - col, a - int(slot_off[w][k]), b - a, k == 0))
            chunks.append((w, col, nc_, segs))
            col += nc_

    # per-core index streams (int16 window-local rows), zero-row = c*SSTRIDE + QC
    for c in range(NCORES):
        cc = cores[c]
        streams = []
        for w in range(4):
            zero_row = c * SSTRIDE + QC
            stream = np.full(WCOLS[w] * P, zero_row, np.int64)
            if WCOLS[w] == 0:
                streams.append(stream.astype(np.int16))
                continue
            sel = cc["e_w"] == w
            ed = cc["e_dst"][sel]
            es = cc["e_srow"][sel]
            rk = cc["rank"][w][ed]
            # slot index k per edge: order by (rank, arrival)
            o = np.argsort(rk, kind="stable")
            rk_s = rk[o]
            es_s = es[o]
            grp_start = np.searchsorted(rk_s, rk_s)  # first pos of each rank value
            kk = np.arange(len(rk_s)) - grp_start
            pos = np.array([int(slot_off[w][k]) for k in range(len(CK[w]))], np.int64)
            stream[pos[kk] * P + rk_s] = es_s
            assert stream.max() < 32768
            streams.append(stream.astype(np.int16))
        cc["streams"] = streams
        # merge stream per window: position n -> staging row rank_w[n]
        cc["merge"] = [cc["rank"][w][:NMAX].astype(np.int16) for w in range(4)]

    # pack idx input: [128, TOTICOL] int16
    # layout: for each chunk (in `chunks` order): icols = ncols*8
    #         then 4 merge streams: NMAXC*8 each
    icol_off = []
    o = 0
    for (w, col, nc_, segs) in chunks:
        icol_off.append(o)
        o += nc_ * 8
    merge_icol = []
    for w in range(4):
        merge_icol.append(o)
        o += NMAXC * 8
    TOTICOL = o

    def pack_stream(seg):  # positions -> [16, len/16] -> replicate to 128 partitions
        a = seg.reshape(-1, 16).T  # [16, n/16]
        return np.tile(a, (8, 1))

    for c in range(NCORES):
        cc = cores[c]
        gidx = np.zeros((P, TOTICOL), np.int16)
        for ci, (w, col, nc_, segs) in enumerate(chunks):
            seg = cc["streams"][w][col * P:(col + nc_) * P]
            gidx[:, icol_off[ci]:icol_off[ci] + nc_ * 8] = pack_stream(seg)
        for w in range(4):
            gidx[:, merge_icol[w]:merge_icol[w] + NMAXC * 8] = pack_stream(cc["merge"][w])
        cc["gidx"] = gidx

    # dinv per core in node order [NMAX] (0 on pads) and in table-slice order [4*SSTRIDE]
    deg = deg_tot_g.astype(np.float64) + 1.0
    for c in range(NCORES):
        cc = cores[c]
        nr = int(n_real[c])
        dv = np.zeros(NMAX, np.float64)
        dv[:nr] = deg[bounds[c]:bounds[c + 1]] ** -0.5
        cc["dinv"] = dv.astype(np.float32)

    # pooling P per core: [NMAXC, 128, GPC] f32 with 1/cnt
    for c in range(NCORES):
        cc = cores[c]
        nr = int(n_real[c])
        bloc = batch[bounds[c]:bounds[c + 1]] - c * GPC
        cnt = np.bincount(bloc, minlength=GPC).astype(np.float64)
        w_ = 1.0 / np.maximum(cnt, 1.0)
        Pm = np.zeros((NMAX, GPC), np.float32)
        Pm[np.arange(nr), bloc] = w_[bloc].astype(np.float32)
        cc["pool"] = Pm.reshape(NMAXC, P, GPC)

    return dict(
        bounds=bounds, n_real=n_real, QC=QC, NMAX=NMAX, NMAXC=NMAXC,
        SSTRIDE=SSTRIDE, WROWS=WROWS, TROWS=4 * WROWS, GPC=GPC,
        CK=CK, slot_off=slot_off, WCOLS=WCOLS, chunks=chunks,
        icol_off=icol_off, merge_icol=merge_icol, TOTICOL=TOTICOL,
        cores=cores,
    )


def make_core_inputs(S, x, weights):
    """Per-core input dicts (numpy). weights: dict W1,b1,W2,b2,W3,b3,Wc1,bc1,Wc2,bc2."""
    NMAX, NMAXC = S["NMAX"], S["NMAXC"]
    bounds = S["bounds"]
    in_maps = []
    for c in range(NCORES):
        cc = S["cores"][c]
        nr = int(S["n_real"][c])
        xT = np.zeros((x.shape[1], NMAX), np.float32)
        xT[:, :nr] = x[bounds[c]:bounds[c + 1]].T
        m = {
            "xT": xT,
            "gidx": cc["gidx"],
            "dinv": cc["dinv"].reshape(NMAXC, P).T.copy(),   # [128, NMAXC]
            "pool": cc["pool"],                               # [NMAXC, 128, GPC]
            "W1": weights["W1"], "W2": weights["W2"], "W3": weights["W3"],
            "Wc1": np.asarray(weights["Wc1"], np.float32),
            "Wc2": np.asarray(weights["Wc2"], np.float32),
            "b1": np.tile(np.asarray(weights["b1"], np.float32)[None, :], (P, 1)),
            "b2": np.tile(np.asarray(weights["b2"], np.float32)[None, :], (P, 1)),
            "b3": np.tile(np.asarray(weights["b3"], np.float32)[None, :], (P, 1)),
            "bc1": np.tile(np.asarray(weights["bc1"], np.float32)[None, :], (P, 1)),
            "bc2": np.tile(np.asarray(weights["bc2"], np.float32)[None, :], (P, 1)),
        }
        in_maps.append(m)
    return in_maps

F_IN = 128
NCLS = 16
FP = mybir.dt.float32
I16 = mybir.dt.int16


def build_program(S, n_cores=8, no_gather=False, no_ccl=False,
                  no_merge=False, no_segs=False, max_chunks=10**9, no_dump=False):
    NMAX, NMAXC = S["NMAX"], S["NMAXC"]
    QC, SSTRIDE, WROWS = S["QC"], S["SSTRIDE"], S["WROWS"]
    QCC = QC // P                      # cols per quarter
    GPC = S["GPC"]
    chunks, icol_off, merge_icol = S["chunks"], S["icol_off"], S["merge_icol"]
    TOTICOL = S["TOTICOL"]

    nc = bacc.Bacc("TRN2", target_bir_lowering=False, debug=False,
                   enable_asserts=True, num_devices=n_cores,
                   num_swdge_queues=4)

    # ---- I/O ----
    xT_in = nc.dram_tensor("xT", [F_IN, NMAX], FP, kind="ExternalInput").ap()
    gidx_in = nc.dram_tensor("gidx", [P, TOTICOL], I16, kind="ExternalInput").ap()
    dinv_in = nc.dram_tensor("dinv", [P, NMAXC], FP, kind="ExternalInput").ap()
    pool_in = nc.dram_tensor("pool", [NMAXC, P, GPC], FP, kind="ExternalInput").ap()
    W1_in = nc.dram_tensor("W1", [F_IN, D], FP, kind="ExternalInput").ap()
    W2_in = nc.dram_tensor("W2", [D, D], FP, kind="ExternalInput").ap()
    W3_in = nc.dram_tensor("W3", [D, 32], FP, kind="ExternalInput").ap()
    Wc1_in = nc.dram_tensor("Wc1", [32, NCLS], FP, kind="ExternalInput").ap()
    Wc2_in = nc.dram_tensor("Wc2", [NCLS, NCLS], FP, kind="ExternalInput").ap()
    b1_in = nc.dram_tensor("b1", [P, D], FP, kind="ExternalInput").ap()
    b2_in = nc.dram_tensor("b2", [P, D], FP, kind="ExternalInput").ap()
    b3_in = nc.dram_tensor("b3", [P, 32], FP, kind="ExternalInput").ap()
    bc1_in = nc.dram_tensor("bc1", [P, NCLS], FP, kind="ExternalInput").ap()
    bc2_in = nc.dram_tensor("bc2", [P, NCLS], FP, kind="ExternalInput").ap()
    out_dram = nc.dram_tensor("out", [GPC, NCLS], FP, kind="ExternalOutput").ap()

    rg = [list(range(n_cores))]

    with tile.TileContext(nc) as tc, ExitStack() as ctx:
        dram = ctx.enter_context(tc.tile_pool(name="dram", bufs=1, space="DRAM"))
        const = ctx.enter_context(tc.tile_pool(name="const", bufs=1))
        sb = ctx.enter_context(tc.tile_pool(name="sb", bufs=1))
        psum = ctx.enter_context(tc.tile_pool(name="psum", bufs=1, space="PSUM"))

        # ---- DRAM internal tensors ----
        tables = [[dram.tile([WROWS, D], FP, name=f"table_l{l}_w{w}", addr_space=("Local" if no_ccl else "Shared"))
                   for w in range(4)] for l in range(3)]
        slices = [[dram.tile([SSTRIDE, D], FP, name=f"slice_l{l}_q{q}")
                   for q in range(4)] for l in range(3)]
        stagings = [[dram.tile([NMAX, D], FP, name=f"stage_l{l}_w{w}")
                     for w in range(4)] for l in range(3)]

        # ---- constants ----
        W1_sb = const.tile([F_IN, D], FP)
        W2_sb = const.tile([D, D], FP)
        W3_sb = const.tile([D, 32], FP)
        Wc1_sb = const.tile([32, NCLS], FP)
        Wc2_sb = const.tile([NCLS, NCLS], FP)
        b1_sb = const.tile([P, D], FP)
        b2_sb = const.tile([P, D], FP)
        b3_sb = const.tile([P, 32], FP)
        bc1_sb = const.tile([P, NCLS], FP)
        bc2_sb = const.tile([P, NCLS], FP)
        dinv_sb = const.tile([P, NMAXC], FP)
        ident = const.tile([P, P], FP)
        zrow = const.tile([P, D], FP)

        nc.sync.dma_start(out=W1_sb[:], in_=W1_in[:, :])
        nc.sync.dma_start(out=W2_sb[:], in_=W2_in[:, :])
        nc.sync.dma_start(out=W3_sb[:], in_=W3_in[:, :])
        nc.sync.dma_start(out=Wc1_sb[:], in_=Wc1_in[:, :])
        nc.sync.dma_start(out=Wc2_sb[:], in_=Wc2_in[:, :])
        nc.sync.dma_start(out=b1_sb[:], in_=b1_in[:, :])
        nc.sync.dma_start(out=b2_sb[:], in_=b2_in[:, :])
        nc.sync.dma_start(out=b3_sb[:], in_=b3_in[:, :])
        nc.sync.dma_start(out=bc1_sb[:], in_=bc1_in[:, :])
        nc.sync.dma_start(out=bc2_sb[:], in_=bc2_in[:, :])
        nc.sync.dma_start(out=dinv_sb[:], in_=dinv_in[:, :])
        make_identity(nc, ident[:])
        nc.vector.memset(zrow[:], 0.0)

        def dinv_b(col):  # [P,1] -> [P,D] broadcast for column col
            return dinv_sb[:, col:col + 1].to_broadcast([P, D])

        # ============ table production stage for layer l ============
        # writes g rows into slices, zero rows, fires allgather -> tables[l]
        def produce_table(l, g_full):
            """g_full: SBUF tile [P, NMAXC, D] = per-node g rows (node order)."""
            for q in range(4):
                nc.sync.dma_start(
                    out=slices[l][q][0:QC, :].rearrange("(c p) d -> p c d", p=P),
                    in_=g_full[:, q * QCC:(q + 1) * QCC, :],
                )
                nc.sync.dma_start(out=slices[l][q][QC:QC + P, :], in_=zrow[:])
                if no_ccl:
                    for cc_ in range(n_cores):
                        nc.sync.dma_start(
                            out=tables[l][q][cc_ * SSTRIDE:(cc_ + 1) * SSTRIDE, :],
                            in_=slices[l][q][:, :])
                else:
                    nc.gpsimd.collective_compute(
                        "AllGather",
                        mybir.AluOpType.bypass,
                        replica_groups=rg,
                        ins=[slices[l][q][:]],
                        outs=[tables[l][q][:]],
                    )

        # ============ matmul stages ============
        def matmul_xW1(xt_pool, mm_ps_pool, g_full):
            # g_full[:, col, :] = dinv * (x @ W1) per 128-node column
            for q in range(4):
                xt = xt_pool.tile([F_IN, QC], FP, tag="xt")
                nc.sync.dma_start(out=xt[:], in_=xT_in[:, q * QC:(q + 1) * QC])
                for cq in range(QCC):
                    col = q * QCC + cq
                    mm = mm_ps_pool.tile([P, D], FP, tag="mm", space="PSUM")
                    nc.tensor.matmul(out=mm[:], lhsT=xt[:, cq * P:(cq + 1) * P],
                                     rhs=W1_sb[:], start=True, stop=True)
                    nc.vector.tensor_mul(out=g_full[:, col, :], in0=mm[:], in1=dinv_b(col))

        def matmul_hW(h_full, W_sb, tp_pool, lhs_pool, mm_ps_pool, g_full):
            # g_full[:, col, :] = dinv * (h @ W) per column (h: [P, NMAXC, D])
            for col in range(NMAXC):
                tp = tp_pool.tile([D, P], FP, tag="tp", space="PSUM")
                nc.tensor.transpose(out=tp[:], in_=h_full[:, col, :], identity=ident[:])
                lhs = lhs_pool.tile([D, P], FP, tag="lhs")
                nc.scalar.copy(out=lhs[:], in_=tp[:])
                mm = mm_ps_pool.tile([P, D], FP, tag="mm", space="PSUM")
                nc.tensor.matmul(out=mm[:], lhsT=lhs[:], rhs=W_sb[:], start=True, stop=True)
                nc.vector.tensor_mul(out=g_full[:, col, :], in0=mm[:], in1=dinv_b(col))

        def scale_h(h_full, g_full):
            # g_full = dinv * h  (layer 3 table: no weight)
            nc.vector.tensor_mul(
                out=g_full[:],
                in0=h_full[:],
                in1=dinv_sb[:, :, None].to_broadcast([P, NMAXC, D]),
            )

        # ============ gather stage for layer l -> agg tile ============
        def gather_stage(l, agg_pool, acc_pool, scr_pool, idx_pool):
            if no_gather:
                agg = agg_pool.tile([P, NMAXC, D], FP, tag="agg", name=f"agg_ng{l}")
                nc.vector.memset(agg[:], 0.0)
                return agg
            active = [w for w in range(4) if S["WCOLS"][w] > 0]
            accs = {}
            for w in active:
                accs[w] = acc_pool.tile([P, NMAXC, D], FP, tag="acc", name=f"acc_w{w}")
            # window phases
            for ci, (w, col0, ncc, segs) in enumerate(chunks):
                if ci >= max_chunks:
                    break
                idx_t = idx_pool.tile([P, ncc * 8], I16, tag="idx")
                nc.sync.dma_start(out=idx_t[:], in_=gidx_in[:, icol_off[ci]:icol_off[ci] + ncc * 8])
                scr = scr_pool.tile([P, ncc, D], FP, tag="scr")
                nc.gpsimd.dma_gather(
                    out_ap=scr[:], in_ap=tables[l][w][:, :], idxs_ap=idx_t[:],
                    num_idxs=ncc * P, num_idxs_reg=ncc * P, elem_size=D,
                    single_packet=False, queue_num=ci % 4,
                )
                for (s_col, a_col, n_col, is_copy) in (() if no_segs else segs):
                    dst = accs[w][:, a_col:a_col + n_col, :]
                    srcv = scr[:, s_col:s_col + n_col, :]
                    if is_copy:
                        nc.vector.tensor_copy(out=dst, in_=srcv)
                    else:
                        nc.vector.tensor_add(out=dst, in0=dst, in1=srcv)
            for w in ([] if (no_segs or no_dump) else active):
                nc.sync.dma_start(
                    out=stagings[l][w][:, :].rearrange("(c p) d -> p c d", p=P),
                    in_=accs[w][:],
                )
            # merge
            agg = agg_pool.tile([P, NMAXC, D], FP, tag="agg")
            if no_merge or no_segs or no_dump:
                nc.vector.memset(agg[:], 0.0)
                return agg
            MC = 64
            mq = 0
            for w in active:
                midx = idx_pool.tile([P, NMAXC * 8], I16, tag="midx")
                nc.sync.dma_start(out=midx[:], in_=gidx_in[:, merge_icol[w]:merge_icol[w] + NMAXC * 8])
                for a in range(0, NMAXC, MC):
                    b = min(a + MC, NMAXC)
                    mq += 1
                    if w == active[0]:
                        nc.gpsimd.dma_gather(
                            out_ap=agg[:, a:b, :], in_ap=stagings[l][w][:, :],
                            idxs_ap=midx[:, a * 8:b * 8],
                            num_idxs=(b - a) * P, num_idxs_reg=(b - a) * P, elem_size=D,
                            single_packet=False, queue_num=mq % 4,
                        )
                    else:
                        mscr = scr_pool.tile([P, b - a, D], FP, tag="scr", name=f"mscr_{l}_{w}_{a}")
                        nc.gpsimd.dma_gather(
                            out_ap=mscr[:], in_ap=stagings[l][w][:, :],
                            idxs_ap=midx[:, a * 8:b * 8],
                            num_idxs=(b - a) * P, num_idxs_reg=(b - a) * P, elem_size=D,
                            single_packet=False, queue_num=mq % 4,
                        )
                        nc.vector.tensor_add(out=agg[:, a:b, :], in0=agg[:, a:b, :], in1=mscr[:])
            return agg

        def finish_h(agg, g_full, b_sb):
            # h = relu(dinv*(agg + g_full) + b)   [in place on agg]
            nc.vector.tensor_add(out=agg[:], in0=agg[:], in1=g_full[:])
            nc.vector.tensor_mul(out=agg[:], in0=agg[:],
                                 in1=dinv_sb[:, :, None].to_broadcast([P, NMAXC, D]))
            nc.vector.tensor_add(out=agg[:], in0=agg[:],
                                 in1=b_sb[:, None, :].to_broadcast([P, NMAXC, D]))
            nc.scalar.activation(out=agg[:], in_=agg[:],
                                 func=mybir.ActivationFunctionType.Relu)
            return agg

        # ---- pools with rotation ----
        xt_pool = ctx.enter_context(tc.tile_pool(name="xt", bufs=1))
        lhs_pool = ctx.enter_context(tc.tile_pool(name="lhs", bufs=2))
        scr_pool = ctx.enter_context(tc.tile_pool(name="scr", bufs=3))
        idx_pool = ctx.enter_context(tc.tile_pool(name="idx", bufs=4))
        acc_pool = ctx.enter_context(tc.tile_pool(name="acc", bufs=2))
        agg_pool = ctx.enter_context(tc.tile_pool(name="agg", bufs=1))
        gf_pool = ctx.enter_context(tc.tile_pool(name="gf", bufs=1))
        h4_pool = ctx.enter_context(tc.tile_pool(name="h4", bufs=3))
        pl_pool = ctx.enter_context(tc.tile_pool(name="pl", bufs=2))
        mm_ps = ctx.enter_context(tc.tile_pool(name="mmps", bufs=2, space="PSUM"))
        tp_ps = ctx.enter_context(tc.tile_pool(name="tpps", bufs=2, space="PSUM"))
        pool_ps = ctx.enter_context(tc.tile_pool(name="poolps", bufs=1, space="PSUM"))

        # ======== Layer 1 ========
        g1 = gf_pool.tile([P, NMAXC, D], FP, tag="gf")
        matmul_xW1(xt_pool, mm_ps, g1)
        produce_table(0, g1)
        agg1 = gather_stage(0, agg_pool, acc_pool, scr_pool, idx_pool)
        h2 = finish_h(agg1, g1, b1_sb)

        # ======== Layer 2 ========
        g2 = gf_pool.tile([P, NMAXC, D], FP, tag="gf")
        matmul_hW(h2, W2_sb, tp_ps, lhs_pool, mm_ps, g2)
        produce_table(1, g2)
        agg2 = gather_stage(1, agg_pool, acc_pool, scr_pool, idx_pool)
        h3 = finish_h(agg2, g2, b2_sb)

        # ======== Layer 3 ========
        g3 = gf_pool.tile([P, NMAXC, D], FP, tag="gf")
        scale_h(h3, g3)
        produce_table(2, g3)
        agg3 = gather_stage(2, agg_pool, acc_pool, scr_pool, idx_pool)
        # a3 = (dinv * (agg3 + g3)) @ W3 + b3 ; h4 = relu(a3); pooled = P^T h4
        nc.vector.tensor_add(out=agg3[:], in0=agg3[:], in1=g3[:])
        nc.vector.tensor_mul(out=agg3[:], in0=agg3[:],
                             in1=dinv_sb[:, :, None].to_broadcast([P, NMAXC, D]))
        pooled_ps = pool_ps.tile([GPC, 32], FP, tag="poolps", space="PSUM")
        for col in range(NMAXC):
            tp = tp_ps.tile([D, P], FP, tag="tp", space="PSUM")
            nc.tensor.transpose(out=tp[:], in_=agg3[:, col, :], identity=ident[:])
            lhs = lhs_pool.tile([D, P], FP, tag="lhs")
            nc.scalar.copy(out=lhs[:], in_=tp[:])
            mm = mm_ps.tile([P, 32], FP, tag="mm", space="PSUM")
            nc.tensor.matmul(out=mm[:], lhsT=lhs[:], rhs=W3_sb[:], start=True, stop=True)
            h4 = h4_pool.tile([P, 32], FP, tag="h4")
            nc.vector.tensor_add(out=h4[:], in0=mm[:], in1=b3_sb[:])
            nc.scalar.activation(out=h4[:], in_=h4[:],
                                 func=mybir.ActivationFunctionType.Relu)
            pl = pl_pool.tile([P, GPC], FP, tag="pl")
            nc.sync.dma_start(out=pl[:], in_=pool_in[col, :, :])
            nc.tensor.matmul(out=pooled_ps[:], lhsT=pl[:], rhs=h4[:],
                             start=(col == 0), stop=(col == NMAXC - 1))

        # ======== head ========
        pooled = sb.tile([GPC, 32], FP)
        nc.scalar.copy(out=pooled[:], in_=pooled_ps[:])
        ptp = tp_ps.tile([32, GPC], FP, tag="tp", space="PSUM")
        nc.tensor.transpose(out=ptp[:], in_=pooled[:], identity=ident[:GPC, :GPC])
        pooledT = sb.tile([32, GPC], FP)
        nc.scalar.copy(out=pooledT[:], in_=ptp[:])
        z_ps = mm_ps.tile([GPC, NCLS], FP, tag="mm", space="PSUM")
        nc.tensor.matmul(out=z_ps[:], lhsT=pooledT[:], rhs=Wc1_sb[:], start=True, stop=True)
        z = sb.tile([GPC, NCLS], FP)
        nc.vector.tensor_add(out=z[:], in0=z_ps[:], in1=bc1_sb[:GPC, :])
        nc.scalar.activation(out=z[:], in_=z[:], func=mybir.ActivationFunctionType.Relu)
        ztp = tp_ps.tile([NCLS, GPC], FP, tag="tp", space="PSUM")
        nc.tensor.transpose(out=ztp[:], in_=z[:], identity=ident[:GPC, :GPC])
        zT = sb.tile([NCLS, GPC], FP)
        nc.scalar.copy(out=zT[:], in_=ztp[:])
        lg_ps = mm_ps.tile([GPC, NCLS], FP, tag="mm", space="PSUM")
        nc.tensor.matmul(out=lg_ps[:], lhsT=zT[:], rhs=Wc2_sb[:], start=True, stop=True)
        logits = sb.tile([GPC, NCLS], FP)
        nc.vector.tensor_add(out=logits[:], in0=lg_ps[:], in1=bc2_sb[:GPC, :])
        # log_softmax
        mx = sb.tile([GPC, 1], FP)
        nc.vector.reduce_max(mx[:], logits[:], axis=mybir.AxisListType.X)
        shifted = sb.tile([GPC, NCLS], FP)
        nc.vector.tensor_tensor(out=shifted[:], in0=logits[:],
                                in1=mx[:].to_broadcast([GPC, NCLS]),
                                op=mybir.AluOpType.subtract)
        ex = sb.tile([GPC, NCLS], FP)
        nc.scalar.activation(out=ex[:], in_=shifted[:], func=mybir.ActivationFunctionType.Exp)
        sm = sb.tile([GPC, 1], FP)
        nc.vector.reduce_sum(sm[:], ex[:], axis=mybir.AxisListType.X)
        lsm = sb.tile([GPC, 1], FP)
        nc.scalar.activation(out=lsm[:], in_=sm[:], func=mybir.ActivationFunctionType.Ln)
        res = sb.tile([GPC, NCLS], FP)
        nc.vector.tensor_tensor(out=res[:], in0=shifted[:],
                                in1=lsm[:].to_broadcast([GPC, NCLS]),
                                op=mybir.AluOpType.subtract)
        nc.sync.dma_start(out=out_dram[:, :], in_=res[:])

    nc.compile()
    return nc


# ======================= public entry point =======================

N_NODES = 100000
N_EDGES = 3200000
N_GRAPHS = 1024
TRACE = False          # set True (with an NTFF hook installed) to print HW time
LAST_EXEC_NS = None


def kernel(x, edge_index, batch, W1, b1, W2, b2, W3, b3, Wc1, bc1, Wc2, bc2):
    x = np.ascontiguousarray(np.asarray(x, np.float32))
    edge_index = np.asarray(edge_index)
    batch = np.asarray(batch)
    weights = dict(W1=np.asarray(W1, np.float32), b1=b1, W2=np.asarray(W2, np.float32),
                   b2=b2, W3=np.asarray(W3, np.float32), b3=b3,
                   Wc1=Wc1, bc1=bc1, Wc2=Wc2, bc2=bc2)

    S = build_schedule(edge_index, batch, N_GRAPHS)
    in_maps = make_core_inputs(S, x, weights)
    nc = build_program(S)
    res = run_bass_kernel_spmd(nc, in_maps, core_ids=list(range(NCORES)), trace=TRACE)
    global LAST_EXEC_NS
    LAST_EXEC_NS = res.exec_time_ns
    out = np.concatenate([res.results[c]["out"] for c in range(NCORES)], 0)
    return out.astype(np.float32)



# revision 8
# speedup vs baseline: 1.0250x; 1.0250x over previous
"""Trainium2 Bass kernel for a 3-layer GCN + mean-pool + MLP head (8 NeuronCores).

Strategy:
  - shard graphs (and their contiguous node ranges) across 8 cores
  - per layer: g = dinv * (h @ W) produced per-core, exchanged via 4 chunked
    AllGathers into a replicated table; aggregation = dma_gather of 256B rows
    with a per-(core,window) degree-sorted prefix-slot schedule; window
    partials merged by 4 small local gathers; W3 applied post-aggregation
  - pooling via PE matmul with a host-built (1/count) one-hot matrix
"""
import math
import sys
from contextlib import ExitStack

sys.path.insert(0, "/opt/trn_rl_repo")

import numpy as np

import concourse.bass as bass
import concourse.bacc as bacc
import concourse.mybir as mybir
import concourse.tile as tile
from concourse.bass_utils import run_bass_kernel_spmd
from concourse.masks import make_identity



P = 128
NCORES = 8
D = 64          # gather row width (f32) = 256B
CHUNK_COLS = 64   # gather chunk size in columns (64*128 = 8192 idxs)


def _ceil(a, b):
    return -(-a // b)


def build_schedule(edge_index, batch, n_graphs=1024):
    """All host-side preprocessing. Returns dict of constants + per-core arrays."""
    src_g = np.asarray(edge_index[0], dtype=np.int64)
    dst_g = np.asarray(edge_index[1], dtype=np.int64)
    batch = np.asarray(batch, dtype=np.int64)
    n_nodes = batch.shape[0]
    GPC = n_graphs // NCORES

    bounds = np.searchsorted(batch, np.arange(NCORES + 1) * GPC)
    n_real = np.diff(bounds)
    QC = 128 * _ceil(_ceil(int(n_real.max()), 4), 128)
    NMAX = 4 * QC
    NMAXC = NMAX // P
    SSTRIDE = QC + 128
    WROWS = 8 * SSTRIDE
    assert WROWS <= 32768

    # source node -> (owner core, local n) -> window + window-local row
    owner = np.searchsorted(bounds, src_g, side="right") - 1  # [E]
    loc = src_g - bounds[owner]
    q_src = loc // QC
    r_src = loc % QC
    src_window = q_src.astype(np.int64)                      # [E] 0..3
    src_wrow = owner * SSTRIDE + r_src                       # window-local row

    # per-core edge partitions by dst
    dst_owner = np.searchsorted(bounds, dst_g, side="right") - 1
    deg_tot_g = np.bincount(dst_g, minlength=n_nodes)

    cores = []
    for c in range(NCORES):
        m = dst_owner == c
        e_dst = (dst_g[m] - bounds[c]).astype(np.int64)   # local dst
        e_w = src_window[m]
        e_srow = src_wrow[m]
        deg_w = np.zeros((4, NMAX), np.int64)
        for w in range(4):
            np.add.at(deg_w[w], e_dst[e_w == w], 1)
        cores.append(dict(e_dst=e_dst, e_w=e_w, e_srow=e_srow, deg_w=deg_w))

    # per-(core, window) orderings and slot counts
    for c in range(NCORES):
        cc = cores[c]
        cc["order"] = []
        cc["rank"] = []
        cc["slotcnt"] = []
        for w in range(4):
            dw = cc["deg_w"][w]
            order = np.argsort(-dw, kind="stable")
            rank = np.empty(NMAX, np.int64)
            rank[order] = np.arange(NMAX)
            cc["order"].append(order)
            cc["rank"].append(rank)
            kmax = int(dw.max())
            cnt = np.array([(dw > k).sum() for k in range(kmax)], np.int64)
            cc["slotcnt"].append(cnt)

    # union slot schedule per window: CK[w][k] in columns
    CK = []
    for w in range(4):
        kmax = max(len(cores[c]["slotcnt"][w]) for c in range(NCORES))
        cols = []
        for k in range(kmax):
            mx = max(
                int(cores[c]["slotcnt"][w][k]) if k < len(cores[c]["slotcnt"][w]) else 0
                for c in range(NCORES)
            )
            cols.append(NMAXC if k == 0 else _ceil(mx, P))
        CK.append(cols)

    # window stream layout: slot k occupies columns [slot_off[w][k], +CK[w][k])
    slot_off = []
    WCOLS = []
    for w in range(4):
        off = np.concatenate([[0], np.cumsum(CK[w])]).astype(np.int64)
        slot_off.append(off)
        WCOLS.append(int(off[-1]))

    # chunk list per window: (col_off, ncols, segments)
    # segment: (scr_col, acc_col, ncols, is_copy)  [is_copy when slot==0 segment]
    chunks = []
    for w in range(4):
        col = 0
        while col < WCOLS[w]:
            nc_ = min(CHUNK_COLS, WCOLS[w] - col)
            segs = []
            # which slots intersect [col, col+nc_)
            for k in range(len(CK[w])):
                a = max(col, int(slot_off[w][k]))
                b = min(col + nc_, int(slot_off[w][k + 1]))
                if a < b:
                    segs.append((a - col, a - int(slot_off[w][k]), b - a, k == 0))
            chunks.append((w, col, nc_, segs))
            col += nc_

    # per-core index streams (int16 window-local rows), zero-row = c*SSTRIDE + QC
    for c in range(NCORES):
        cc = cores[c]
        streams = []
        for w in range(4):
            zero_row = c * SSTRIDE + QC
            stream = np.full(WCOLS[w] * P, zero_row, np.int64)
            if WCOLS[w] == 0:
                streams.append(stream.astype(np.int16))
                continue
            sel = cc["e_w"] == w
            ed = cc["e_dst"][sel]
            es = cc["e_srow"][sel]
            rk = cc["rank"][w][ed]
            # slot index k per edge: order by (rank, arrival)
            o = np.argsort(rk, kind="stable")
            rk_s = rk[o]
            es_s = es[o]
            grp_start = np.searchsorted(rk_s, rk_s)  # first pos of each rank value
            kk = np.arange(len(rk_s)) - grp_start
            pos = np.array([int(slot_off[w][k]) for k in range(len(CK[w]))], np.int64)
            stream[pos[kk] * P + rk_s] = es_s
            assert stream.max() < 32768
            streams.append(stream.astype(np.int16))
        cc["streams"] = streams
        # merge stream per window: position n -> staging row rank_w[n]
        cc["merge"] = [cc["rank"][w][:NMAX].astype(np.int16) for w in range(4)]

    # pack idx input: [128, TOTICOL] int16
    # layout: for each chunk (in `chunks` order): icols = ncols*8
    #         then 4 merge streams: NMAXC*8 each
    icol_off = []
    o = 0
    for (w, col, nc_, segs) in chunks:
        icol_off.append(o)
        o += nc_ * 8
    merge_icol = []
    for w in range(4):
        merge_icol.append(o)
        o += NMAXC * 8
    TOTICOL = o

    def pack_stream(seg):  # positions -> [16, len/16] -> replicate to 128 partitions
        a = seg.reshape(-1, 16).T  # [16, n/16]
        return np.tile(a, (8, 1))

    for c in range(NCORES):
        cc = cores[c]
        gidx = np.zeros((P, TOTICOL), np.int16)
        for ci, (w, col, nc_, segs) in enumerate(chunks):
            seg = cc["streams"][w][col * P:(col + nc_) * P]
            gidx[:, icol_off[ci]:icol_off[ci] + nc_ * 8] = pack_stream(seg)
        for w in range(4):
            gidx[:, merge_icol[w]:merge_icol[w] + NMAXC * 8] = pack_stream(cc["merge"][w])
        cc["gidx"] = gidx

    # dinv per core in node order [NMAX] (0 on pads) and in table-slice order [4*SSTRIDE]
    deg = deg_tot_g.astype(np.float64) + 1.0
    for c in range(NCORES):
        cc = cores[c]
        nr = int(n_real[c])
        dv = np.zeros(NMAX, np.float64)
        dv[:nr] = deg[bounds[c]:bounds[c + 1]] ** -0.5
        cc["dinv"] = dv.astype(np.float32)

    # pooling P per core: [NMAXC, 128, GPC] f32 with 1/cnt
    for c in range(NCORES):
        cc = cores[c]
        nr = int(n_real[c])
        bloc = batch[bounds[c]:bounds[c + 1]] - c * GPC
        cnt = np.bincount(bloc, minlength=GPC).astype(np.float64)
        w_ = 1.0 / np.maximum(cnt, 1.0)
        Pm = np.zeros((NMAX, GPC), np.float32)
        Pm[np.arange(nr), bloc] = w_[bloc].astype(np.float32)
        cc["pool"] = Pm.reshape(NMAXC, P, GPC)

    return dict(
        bounds=bounds, n_real=n_real, QC=QC, NMAX=NMAX, NMAXC=NMAXC,
        SSTRIDE=SSTRIDE, WROWS=WROWS, TROWS=4 * WROWS, GPC=GPC,
        CK=CK, slot_off=slot_off, WCOLS=WCOLS, chunks=chunks,
        icol_off=icol_off, merge_icol=merge_icol, TOTICOL=TOTICOL,
        cores=cores,
    )


def make_core_inputs(S, x, weights):
    """Per-core input dicts (numpy). weights: dict W1,b1,W2,b2,W3,b3,Wc1,bc1,Wc2,bc2."""
    NMAX, NMAXC = S["NMAX"], S["NMAXC"]
    bounds = S["bounds"]
    in_maps = []
    for c in range(NCORES):
        cc = S["cores"][c]
        nr = int(S["n_real"][c])
        xT = np.zeros((x.shape[1], NMAX), np.float32)
        xT[:, :nr] = x[bounds[c]:bounds[c + 1]].T
        m = {
            "xT": xT,
            "gidx": cc["gidx"],
            "dinv": cc["dinv"].reshape(NMAXC, P).T.copy(),   # [128, NMAXC]
            "pool": cc["pool"],                               # [NMAXC, 128, GPC]
            "W1": weights["W1"], "W2": weights["W2"], "W3": weights["W3"],
            "Wc1": np.asarray(weights["Wc1"], np.float32),
            "Wc2": np.asarray(weights["Wc2"], np.float32),
            "b1": np.tile(np.asarray(weights["b1"], np.float32)[None, :], (P, 1)),
            "b2": np.tile(np.asarray(weights["b2"], np.float32)[None, :], (P, 1)),
            "b3": np.tile(np.asarray(weights["b3"], np.float32)[None, :], (P, 1)),
            "bc1": np.tile(np.asarray(weights["bc1"], np.float32)[None, :], (P, 1)),
            "bc2": np.tile(np.asarray(weights["bc2"], np.float32)[None, :], (P, 1)),
        }
        in_maps.append(m)
    return in_maps

F_IN = 128
NCLS = 16
FP = mybir.dt.float32
I16 = mybir.dt.int16


def build_program(S, n_cores=8, no_gather=False, no_ccl=False,
                  no_merge=False, no_segs=False, max_chunks=10**9, no_dump=False):
    NMAX, NMAXC = S["NMAX"], S["NMAXC"]
    QC, SSTRIDE, WROWS = S["QC"], S["SSTRIDE"], S["WROWS"]
    QCC = QC // P                      # cols per quarter
    GPC = S["GPC"]
    chunks, icol_off, merge_icol = S["chunks"], S["icol_off"], S["merge_icol"]
    TOTICOL = S["TOTICOL"]

    nc = bacc.Bacc("TRN2", target_bir_lowering=False, debug=False,
                   enable_asserts=True, num_devices=n_cores,
                   num_swdge_queues=4)

    # ---- I/O ----
    xT_in = nc.dram_tensor("xT", [F_IN, NMAX], FP, kind="ExternalInput").ap()
    gidx_in = nc.dram_tensor("gidx", [P, TOTICOL], I16, kind="ExternalInput").ap()
    dinv_in = nc.dram_tensor("dinv", [P, NMAXC], FP, kind="ExternalInput").ap()
    pool_in = nc.dram_tensor("pool", [NMAXC, P, GPC], FP, kind="ExternalInput").ap()
    W1_in = nc.dram_tensor("W1", [F_IN, D], FP, kind="ExternalInput").ap()
    W2_in = nc.dram_tensor("W2", [D, D], FP, kind="ExternalInput").ap()
    W3_in = nc.dram_tensor("W3", [D, 32], FP, kind="ExternalInput").ap()
    Wc1_in = nc.dram_tensor("Wc1", [32, NCLS], FP, kind="ExternalInput").ap()
    Wc2_in = nc.dram_tensor("Wc2", [NCLS, NCLS], FP, kind="ExternalInput").ap()
    b1_in = nc.dram_tensor("b1", [P, D], FP, kind="ExternalInput").ap()
    b2_in = nc.dram_tensor("b2", [P, D], FP, kind="ExternalInput").ap()
    b3_in = nc.dram_tensor("b3", [P, 32], FP, kind="ExternalInput").ap()
    bc1_in = nc.dram_tensor("bc1", [P, NCLS], FP, kind="ExternalInput").ap()
    bc2_in = nc.dram_tensor("bc2", [P, NCLS], FP, kind="ExternalInput").ap()
    out_dram = nc.dram_tensor("out", [GPC, NCLS], FP, kind="ExternalOutput").ap()

    rg = [list(range(n_cores))]

    with tile.TileContext(nc) as tc, ExitStack() as ctx:
        dram = ctx.enter_context(tc.tile_pool(name="dram", bufs=1, space="DRAM"))
        const = ctx.enter_context(tc.tile_pool(name="const", bufs=1))
        sb = ctx.enter_context(tc.tile_pool(name="sb", bufs=1))
        psum = ctx.enter_context(tc.tile_pool(name="psum", bufs=1, space="PSUM"))

        # ---- DRAM internal tensors ----
        tables = [[dram.tile([WROWS, D], FP, name=f"table_l{l}_w{w}", addr_space=("Local" if no_ccl else "Shared"))
                   for w in range(4)] for l in range(3)]
        slices = [[dram.tile([SSTRIDE, D], FP, name=f"slice_l{l}_q{q}")
                   for q in range(4)] for l in range(3)]
        stagings = [[dram.tile([NMAX, D], FP, name=f"stage_l{l}_w{w}")
                     for w in range(4)] for l in range(3)]

        # ---- constants ----
        W1_sb = const.tile([F_IN, D], FP)
        W2_sb = const.tile([D, D], FP)
        W3_sb = const.tile([D, 32], FP)
        Wc1_sb = const.tile([32, NCLS], FP)
        Wc2_sb = const.tile([NCLS, NCLS], FP)
        b1_sb = const.tile([P, D], FP)
        b2_sb = const.tile([P, D], FP)
        b3_sb = const.tile([P, 32], FP)
        bc1_sb = const.tile([P, NCLS], FP)
        bc2_sb = const.tile([P, NCLS], FP)
        dinv_sb = const.tile([P, NMAXC], FP)
        ident = const.tile([P, P], FP)
        zrow = const.tile([P, D], FP)

        nc.sync.dma_start(out=W1_sb[:], in_=W1_in[:, :])
        nc.sync.dma_start(out=W2_sb[:], in_=W2_in[:, :])
        nc.sync.dma_start(out=W3_sb[:], in_=W3_in[:, :])
        nc.sync.dma_start(out=Wc1_sb[:], in_=Wc1_in[:, :])
        nc.sync.dma_start(out=Wc2_sb[:], in_=Wc2_in[:, :])
        nc.sync.dma_start(out=b1_sb[:], in_=b1_in[:, :])
        nc.sync.dma_start(out=b2_sb[:], in_=b2_in[:, :])
        nc.sync.dma_start(out=b3_sb[:], in_=b3_in[:, :])
        nc.sync.dma_start(out=bc1_sb[:], in_=bc1_in[:, :])
        nc.sync.dma_start(out=bc2_sb[:], in_=bc2_in[:, :])
        nc.sync.dma_start(out=dinv_sb[:], in_=dinv_in[:, :])
        make_identity(nc, ident[:])
        nc.vector.memset(zrow[:], 0.0)

        def dinv_b(col):  # [P,1] -> [P,D] broadcast for column col
            return dinv_sb[:, col:col + 1].to_broadcast([P, D])

        # ============ table production stage for layer l ============
        # writes g rows into slices, zero rows, fires allgather -> tables[l]
        def produce_table_q(l, g_full, q):
            nc.sync.dma_start(
                out=slices[l][q][0:QC, :].rearrange("(c p) d -> p c d", p=P),
                in_=g_full[:, q * QCC:(q + 1) * QCC, :],
            )
            nc.sync.dma_start(out=slices[l][q][QC:QC + P, :], in_=zrow[:])
            if no_ccl:
                for cc_ in range(n_cores):
                    nc.sync.dma_start(
                        out=tables[l][q][cc_ * SSTRIDE:(cc_ + 1) * SSTRIDE, :],
                        in_=slices[l][q][:, :])
            else:
                nc.gpsimd.collective_compute(
                    "AllGather",
                    mybir.AluOpType.bypass,
                    replica_groups=rg,
                    ins=[slices[l][q][:]],
                    outs=[tables[l][q][:]],
                )

        def produce_table(l, g_full):
            for q in range(4):
                produce_table_q(l, g_full, q)

        # ============ matmul stages ============
        def matmul_xW1(xt_pool, mm_ps_pool, g_full):
            # g_full[:, col, :] = dinv * (x @ W1) per 128-node column
            for q in range(4):
                xt = xt_pool.tile([F_IN, QC], FP, tag="xt")
                nc.sync.dma_start(out=xt[:], in_=xT_in[:, q * QC:(q + 1) * QC])
                for cq in range(QCC):
                    col = q * QCC + cq
                    mm = mm_ps_pool.tile([P, D], FP, tag="mm", space="PSUM")
                    nc.tensor.matmul(out=mm[:], lhsT=xt[:, cq * P:(cq + 1) * P],
                                     rhs=W1_sb[:], start=True, stop=True)
                    nc.vector.tensor_mul(out=g_full[:, col, :], in0=mm[:], in1=dinv_b(col))

        def matmul_hW(h_full, W_sb, tp_pool, lhs_pool, mm_ps_pool, g_full):
            # g_full[:, col, :] = dinv * (h @ W) per column (h: [P, NMAXC, D])
            for col in range(NMAXC):
                tp = tp_pool.tile([D, P], FP, tag="tp", space="PSUM")
                nc.tensor.transpose(out=tp[:], in_=h_full[:, col, :], identity=ident[:])
                lhs = lhs_pool.tile([D, P], FP, tag="lhs")
                nc.scalar.copy(out=lhs[:], in_=tp[:])
                mm = mm_ps_pool.tile([P, D], FP, tag="mm", space="PSUM")
                nc.tensor.matmul(out=mm[:], lhsT=lhs[:], rhs=W_sb[:], start=True, stop=True)
                nc.vector.tensor_mul(out=g_full[:, col, :], in0=mm[:], in1=dinv_b(col))

        def scale_h(h_full, g_full):
            # g_full = dinv * h  (layer 3 table: no weight)
            nc.vector.tensor_mul(
                out=g_full[:],
                in0=h_full[:],
                in1=dinv_sb[:, :, None].to_broadcast([P, NMAXC, D]),
            )

        # ============ gather stage for layer l -> agg tile ============
        qctr = [0]

        def next_q():
            qctr[0] += 1
            return qctr[0] % 4

        def gather_stage(l, agg_pool, acc_pool, scr_pool, idx_pool, consume=None):
            if no_gather:
                agg = agg_pool.tile([P, NMAXC, D], FP, tag="agg", name=f"agg_ng{l}")
                nc.vector.memset(agg[:], 0.0)
                return agg
            active = [w for w in range(4) if S["WCOLS"][w] > 0]
            accs = {}
            for w in active:
                accs[w] = acc_pool.tile([P, NMAXC, D], FP, tag="acc", name=f"acc_w{w}")
            # window phases
            for ci, (w, col0, ncc, segs) in enumerate(chunks):
                if ci >= max_chunks:
                    break
                idx_t = idx_pool.tile([P, ncc * 8], I16, tag="idx")
                nc.sync.dma_start(out=idx_t[:], in_=gidx_in[:, icol_off[ci]:icol_off[ci] + ncc * 8])
                scr = scr_pool.tile([P, ncc, D], FP, tag="scr")
                nc.gpsimd.dma_gather(
                    out_ap=scr[:], in_ap=tables[l][w][:, :], idxs_ap=idx_t[:],
                    num_idxs=ncc * P, num_idxs_reg=ncc * P, elem_size=D,
                    single_packet=False, queue_num=next_q(),
                )
                for (s_col, a_col, n_col, is_copy) in (() if no_segs else segs):
                    dst = accs[w][:, a_col:a_col + n_col, :]
                    srcv = scr[:, s_col:s_col + n_col, :]
                    if is_copy:
                        nc.vector.tensor_copy(out=dst, in_=srcv)
                    else:
                        nc.vector.tensor_add(out=dst, in0=dst, in1=srcv)
            for w in ([] if (no_segs or no_dump) else active):
                nc.sync.dma_start(
                    out=stagings[l][w][:, :].rearrange("(c p) d -> p c d", p=P),
                    in_=accs[w][:],
                )
            # merge (chunk-outer so each col-range finishes early and can be
            # consumed while later merge gathers still run)
            agg = agg_pool.tile([P, NMAXC, D], FP, tag="agg")
            if no_merge or no_segs or no_dump:
                nc.vector.memset(agg[:], 0.0)
                return agg
            MC = QCC
            for a in range(0, NMAXC, MC):
                b = min(a + MC, NMAXC)
                for w in active:
                    midx = idx_pool.tile([P, (b - a) * 8], I16, tag="idx")
                    nc.sync.dma_start(
                        out=midx[:],
                        in_=gidx_in[:, merge_icol[w] + a * 8:merge_icol[w] + b * 8])
                    if w == active[0]:
                        nc.gpsimd.dma_gather(
                            out_ap=agg[:, a:b, :], in_ap=stagings[l][w][:, :],
                            idxs_ap=midx[:],
                            num_idxs=(b - a) * P, num_idxs_reg=(b - a) * P, elem_size=D,
                            single_packet=False, queue_num=next_q(),
                        )
                    else:
                        mscr = scr_pool.tile([P, b - a, D], FP, tag="scr", name=f"mscr_{l}_{w}_{a}")
                        nc.gpsimd.dma_gather(
                            out_ap=mscr[:], in_ap=stagings[l][w][:, :],
                            idxs_ap=midx[:],
                            num_idxs=(b - a) * P, num_idxs_reg=(b - a) * P, elem_size=D,
                            single_packet=False, queue_num=next_q(),
                        )
                        nc.vector.tensor_add(out=agg[:, a:b, :], in0=agg[:, a:b, :], in1=mscr[:])
                if consume is not None:
                    consume(agg, a, b)
            return agg

        def finish_h(agg, g_full, b_sb):
            # h = relu(dinv*(agg + g_full) + b)   [in place on agg]
            nc.vector.tensor_add(out=agg[:], in0=agg[:], in1=g_full[:])
            nc.vector.tensor_mul(out=agg[:], in0=agg[:],
                                 in1=dinv_sb[:, :, None].to_broadcast([P, NMAXC, D]))
            nc.vector.tensor_add(out=agg[:], in0=agg[:],
                                 in1=b_sb[:, None, :].to_broadcast([P, NMAXC, D]))
            nc.scalar.activation(out=agg[:], in_=agg[:],
                                 func=mybir.ActivationFunctionType.Relu)
            return agg

        # ---- pools with rotation ----
        xt_pool = ctx.enter_context(tc.tile_pool(name="xt", bufs=1))
        lhs_pool = ctx.enter_context(tc.tile_pool(name="lhs", bufs=2))
        scr_pool = ctx.enter_context(tc.tile_pool(name="scr", bufs=3))
        idx_pool = ctx.enter_context(tc.tile_pool(name="idx", bufs=4))
        acc_pool = ctx.enter_context(tc.tile_pool(name="acc", bufs=2))
        agg_pool = ctx.enter_context(tc.tile_pool(name="agg", bufs=1))
        gf_pool = ctx.enter_context(tc.tile_pool(name="gf", bufs=2))
        h4_pool = ctx.enter_context(tc.tile_pool(name="h4", bufs=3))
        pl_pool = ctx.enter_context(tc.tile_pool(name="pl", bufs=2))
        mm_ps = ctx.enter_context(tc.tile_pool(name="mmps", bufs=2, space="PSUM"))
        tp_ps = ctx.enter_context(tc.tile_pool(name="tpps", bufs=2, space="PSUM"))
        pool_ps = ctx.enter_context(tc.tile_pool(name="poolps", bufs=1, space="PSUM"))

        # per-col-range finish: h = relu(dinv*(agg + g) + b), in place on agg
        def finish_cols(agg, g_full, b_sb, a, b):
            nc.vector.tensor_add(out=agg[:, a:b, :], in0=agg[:, a:b, :],
                                 in1=g_full[:, a:b, :])
            nc.vector.tensor_mul(out=agg[:, a:b, :], in0=agg[:, a:b, :],
                                 in1=dinv_sb[:, a:b, None].to_broadcast([P, b - a, D]))
            nc.vector.tensor_add(out=agg[:, a:b, :], in0=agg[:, a:b, :],
                                 in1=b_sb[:, None, :].to_broadcast([P, b - a, D]))
            nc.scalar.activation(out=agg[:, a:b, :], in_=agg[:, a:b, :],
                                 func=mybir.ActivationFunctionType.Relu)

        # per-col-range next-layer g production (g = dinv * (h @ W))
        def gcols_hW(h_full, W_sb, g_full, a, b):
            for col in range(a, b):
                tp = tp_ps.tile([D, P], FP, tag="tp", space="PSUM")
                nc.tensor.transpose(out=tp[:], in_=h_full[:, col, :], identity=ident[:])
                lhs = lhs_pool.tile([D, P], FP, tag="lhs")
                nc.scalar.copy(out=lhs[:], in_=tp[:])
                mm = mm_ps.tile([P, D], FP, tag="mm", space="PSUM")
                nc.tensor.matmul(out=mm[:], lhsT=lhs[:], rhs=W_sb[:], start=True, stop=True)
                nc.vector.tensor_mul(out=g_full[:, col, :], in0=mm[:], in1=dinv_b(col))

        # ======== Layer 1 ========
        g1 = gf_pool.tile([P, NMAXC, D], FP, tag="gf")
        matmul_xW1(xt_pool, mm_ps, g1)
        produce_table(0, g1)

        g2 = gf_pool.tile([P, NMAXC, D], FP, tag="gf")

        def consume1(agg, a, b):
            finish_cols(agg, g1, b1_sb, a, b)
            gcols_hW(agg, W2_sb, g2, a, b)
            if (b % QCC) == 0:
                produce_table_q(1, g2, b // QCC - 1)

        h2 = gather_stage(0, agg_pool, acc_pool, scr_pool, idx_pool, consume=consume1)

        # ======== Layer 2 ========
        g3 = gf_pool.tile([P, NMAXC, D], FP, tag="gf")

        def consume2(agg, a, b):
            finish_cols(agg, g2, b2_sb, a, b)
            nc.vector.tensor_mul(out=g3[:, a:b, :], in0=agg[:, a:b, :],
                                 in1=dinv_sb[:, a:b, None].to_broadcast([P, b - a, D]))
            if (b % QCC) == 0:
                produce_table_q(2, g3, b // QCC - 1)

        h3 = gather_stage(1, agg_pool, acc_pool, scr_pool, idx_pool, consume=consume2)

        # ======== Layer 3 ========
        pooled_ps = pool_ps.tile([GPC, 32], FP, tag="poolps", space="PSUM")

        def consume3(agg, a, b):
            # a3 = (dinv * (agg3 + g3)) @ W3 + b3 ; h4 = relu(a3); pooled = P^T h4
            nc.vector.tensor_add(out=agg[:, a:b, :], in0=agg[:, a:b, :],
                                 in1=g3[:, a:b, :])
            nc.vector.tensor_mul(out=agg[:, a:b, :], in0=agg[:, a:b, :],
                                 in1=dinv_sb[:, a:b, None].to_broadcast([P, b - a, D]))
            for col in range(a, b):
                tp = tp_ps.tile([D, P], FP, tag="tp", space="PSUM")
                nc.tensor.transpose(out=tp[:], in_=agg[:, col, :], identity=ident[:])
                lhs = lhs_pool.tile([D, P], FP, tag="lhs")
                nc.scalar.copy(out=lhs[:], in_=tp[:])
                mm = mm_ps.tile([P, 32], FP, tag="mm", space="PSUM")
                nc.tensor.matmul(out=mm[:], lhsT=lhs[:], rhs=W3_sb[:], start=True, stop=True)
                h4 = h4_pool.tile([P, 32], FP, tag="h4")
                nc.vector.tensor_add(out=h4[:], in0=mm[:], in1=b3_sb[:])
                nc.scalar.activation(out=h4[:], in_=h4[:],
                                     func=mybir.ActivationFunctionType.Relu)
                pl = pl_pool.tile([P, GPC], FP, tag="pl")
                nc.sync.dma_start(out=pl[:], in_=pool_in[col, :, :])
                nc.tensor.matmul(out=pooled_ps[:], lhsT=pl[:], rhs=h4[:],
                                 start=(col == 0), stop=(col == NMAXC - 1))

        agg3 = gather_stage(2, agg_pool, acc_pool, scr_pool, idx_pool, consume=consume3)

        # ======== head ========
        pooled = sb.tile([GPC, 32], FP)
        nc.scalar.copy(out=pooled[:], in_=pooled_ps[:])
        ptp = tp_ps.tile([32, GPC], FP, tag="tp", space="PSUM")
        nc.tensor.transpose(out=ptp[:], in_=pooled[:], identity=ident[:GPC, :GPC])
        pooledT = sb.tile([32, GPC], FP)
        nc.scalar.copy(out=pooledT[:], in_=ptp[:])
        z_ps = mm_ps.tile([GPC, NCLS], FP, tag="mm", space="PSUM")
        nc.tensor.matmul(out=z_ps[:], lhsT=pooledT[:], rhs=Wc1_sb[:], start=True, stop=True)
        z = sb.tile([GPC, NCLS], FP)
        nc.vector.tensor_add(out=z[:], in0=z_ps[:], in1=bc1_sb[:GPC, :])
        nc.scalar.activation(out=z[:], in_=z[:], func=mybir.ActivationFunctionType.Relu)
        ztp = tp_ps.tile([NCLS, GPC], FP, tag="tp", space="PSUM")
        nc.tensor.transpose(out=ztp[:], in_=z[:], identity=ident[:GPC, :GPC])
        zT = sb.tile([NCLS, GPC], FP)
        nc.scalar.copy(out=zT[:], in_=ztp[:])
        lg_ps = mm_ps.tile([GPC, NCLS], FP, tag="mm", space="PSUM")
        nc.tensor.matmul(out=lg_ps[:], lhsT=zT[:], rhs=Wc2_sb[:], start=True, stop=True)
        logits = sb.tile([GPC, NCLS], FP)
        nc.vector.tensor_add(out=logits[:], in0=lg_ps[:], in1=bc2_sb[:GPC, :])
        # log_softmax
        mx = sb.tile([GPC, 1], FP)
        nc.vector.reduce_max(mx[:], logits[:], axis=mybir.AxisListType.X)
        shifted = sb.tile([GPC, NCLS], FP)
        nc.vector.tensor_tensor(out=shifted[:], in0=logits[:],
                                in1=mx[:].to_broadcast([GPC, NCLS]),
                                op=mybir.AluOpType.subtract)
        ex = sb.tile([GPC, NCLS], FP)
        nc.scalar.activation(out=ex[:], in_=shifted[:], func=mybir.ActivationFunctionType.Exp)
        sm = sb.tile([GPC, 1], FP)
        nc.vector.reduce_sum(sm[:], ex[:], axis=mybir.AxisListType.X)
        lsm = sb.tile([GPC, 1], FP)
        nc.scalar.activation(out=lsm[:], in_=sm[:], func=mybir.ActivationFunctionType.Ln)
        res = sb.tile([GPC, NCLS], FP)
        nc.vector.tensor_tensor(out=res[:], in0=shifted[:],
                                in1=lsm[:].to_broadcast([GPC, NCLS]),
                                op=mybir.AluOpType.subtract)
        nc.sync.dma_start(out=out_dram[:, :], in_=res[:])

    nc.compile()
    return nc


# ======================= public entry point =======================

N_NODES = 100000
N_EDGES = 3200000
N_GRAPHS = 1024
TRACE = False          # set True (with an NTFF hook installed) to print HW time
LAST_EXEC_NS = None


def kernel(x, edge_index, batch, W1, b1, W2, b2, W3, b3, Wc1, bc1, Wc2, bc2):
    x = np.ascontiguousarray(np.asarray(x, np.float32))
    edge_index = np.asarray(edge_index)
    batch = np.asarray(batch)
    weights = dict(W1=np.asarray(W1, np.float32), b1=b1, W2=np.asarray(W2, np.float32),
                   b2=b2, W3=np.asarray(W3, np.float32), b3=b3,
                   Wc1=Wc1, bc1=bc1, Wc2=Wc2, bc2=bc2)

    S = build_schedule(edge_index, batch, N_GRAPHS)
    in_maps = make_core_inputs(S, x, weights)
    nc = build_program(S)
    res = run_bass_kernel_spmd(nc, in_maps, core_ids=list(range(NCORES)), trace=TRACE)
    global LAST_EXEC_NS
    LAST_EXEC_NS = res.exec_time_ns
    out = np.concatenate([res.results[c]["out"] for c in range(NCORES)], 0)
    return out.astype(np.float32)

